# revision 14
# baseline (speedup 1.0000x reference)
"""Trainium2 Bass kernel for nn_DiscriminationLoss (segment_reduce).

v6 design (8 NeuronCores, pixel-sharded; full inputs in, full loss out):

  - HOST packs pred into the PE stationary layout directly: fp16,
    pre-scaled by 2^14, block-diagonal unit slabs [p, (u, b, c)] with a
    baked-in ones column at c == 8 (per-kernel counts).  Numerically
    identical to the on-chip ScalarE cast the v2 kernel used, but
    removes that whole stage and halves pred DMA to 9.4 MB/core.  Each
    512-block-col DMA group is one contiguous 1.18 MB HBM read.
  - Labels ship as bf16 (0..32 exact).  fp16 tensor_scalar inputs take
    a ~13x slow path on DVE; bf16-in -> fp16-out is the fast combo.
  - One-hot is class-major [p, (g, t)]: every DVE tensor_scalar
    (is_equal) writes a dense step-1 16-bit AP (4x_2p mode, ~194 ns
    per [128, 512] op).  The matmul moving operand reads it via a 2D
    AP (g: stride FC, b: 4 dense) at the usual 1 col/cycle.
  - DVE alone cannot cover 32 classes under the DMA floor, and GpSimd
    tensor ops are banned (they serialize the DVE via an SBUF port
    mutex: DVE TS ops stall for the entire concurrent GpSimd op).  So
    ScalarE covers labels 28..32 via the RAMP trick: moving groups
    27..33 hold |lab - r| (one ACTIVATE Abs per ramp, exact in fp16),
    and the host recovers S[k] = (A[k-1] - 2A[k] + A[k+1]) / 2 -- the
    second difference of abs-ramps is exactly the one-hot.  Counts for
    those labels fall out of the ones row the same way.
  - 8 chunks of 512 block-cols keep the pipeline fill/tail short
    (chunk-0's one-hot serializes before the first real matmul).
  - PE: two concurrent 64-col tiles (t = u%2), separate PSUM banks,
    [128, 36]^T @ [128, 136] per unit, PSUM-accumulated over 512 units
    per tile.
  - 36 warmup matmuls trip the PE HAM clock gate during the DMA fill
    (more would block the real stream: the Tensor queue is in-order
    and cold MMs run 107 ns apiece).
  - Host sums per-core partials (the "psum" step of the sharding hint)
    and evaluates the tiny O(K^2) pairwise tail in f64.
"""

import sys
import functools

sys.path.insert(0, "/opt/trn_rl_repo")

import numpy as np

C = 8
K = 32
NCORES = 8
H = W = 2048
PTOT = H * W
PCORE = PTOT // NCORES  # 524288
SIGMA_DIS = 3.0
PRED_SCALE = float(2.0**14)

QB = 2            # pixel-blocks per matmul unit (block-diagonal trick)
NCH = C + 1       # 8 pred channels + ones column (counts)
NSTAT = NCH * QB  # stationary columns per unit = 18 (fits a 32-col tile)
NTILE = 4         # concurrent 32-col PE tiles
DVE_K = 26        # direct one-hot classes (labels 1..26) on DVE
NRAMP = 8         # abs-ramp groups, centers 26..33, on ScalarE
NG = DVE_K + NRAMP  # moving groups per unit = 34
NMOV = NG * QB    # moving columns per unit = 68
FTOT = PCORE // 128  # 4096 block-cols
FG = 512          # block-cols per pred DMA group (1.18 MB contiguous)
FC = 1024         # block-cols per one-hot chunk
NGROUPS = FTOT // FG
NCHUNKS = FTOT // FC
NUNITS = FTOT // QB
SLAB_BUFS = 6
WARM_MMS = 36
LAB_SIZES = (512, 512, 1024, 2048)  # label DMA split (block-cols)


def build_nc():
    import concourse.bacc as bacc
    import concourse.tile as tile
    import concourse.mybir as mybir
    from contextlib import ExitStack

    f32 = mybir.dt.float32
    fp16 = mybir.dt.float16
    bf16 = mybir.dt.bfloat16

    nc = bacc.Bacc(
        "TRN2", target_bir_lowering=False, debug=False, num_devices=NCORES
    )
    # Group-major pred slabs: row block g*128..g*128+127 is DMA group g,
    # a single contiguous 1.18 MB HBM region.
    pred_ext = nc.dram_tensor(
        "pred", [NGROUPS * 128, FG // QB * NSTAT], fp16, kind="ExternalInput"
    )
    lab_ext = nc.dram_tensor("labels", [128, FTOT], bf16, kind="ExternalInput")
    out_ext = nc.dram_tensor(
        "out_s", [128, NMOV + 8], f32, kind="ExternalOutput"
    )

    with tile.TileContext(nc) as tc, ExitStack() as ctx:
        const_pool = ctx.enter_context(tc.tile_pool(name="const", bufs=1))
        slab_pool = ctx.enter_context(
            tc.tile_pool(name="slab", bufs=SLAB_BUFS)
        )
        oh_pool = ctx.enter_context(tc.tile_pool(name="oh", bufs=2))
        psum_pool = ctx.enter_context(
            tc.tile_pool(name="psum", bufs=1, space="PSUM")
        )

        labf = const_pool.tile([128, FTOT], bf16)
        outt = const_pool.tile([128, NMOV + 8], f32)
        warm_t = const_pool.tile([128, 128], bf16)
        # ramp bias constants: -(center) for centers 27..33
        bias_t = const_pool.tile([128, NRAMP], f32)
        for i in range(NRAMP):
            nc.vector.memset(bias_t[:, i : i + 1], -float(DVE_K + i))

        psums = [
            psum_pool.tile([128, 512], f32, name=f"psum_t{t}")
            for t in range(NTILE)
        ]
        warm_ps = psum_pool.tile([128, 128], f32)

        # ---- DMA: labels + pred interleaved on the sync ring, labels
        # first so chunk-0's one-hot can start ASAP.
        slabs = []

        def emit_pred_dma(g):
            st = slab_pool.tile([128, FG // QB * NSTAT], fp16, tag="slab")
            nc.sync.dma_start(st, pred_ext[g * 128 : (g + 1) * 128, :])
            slabs.append(st)

        lab_off = 0
        next_dma = 0
        for lg in LAB_SIZES:
            nc.sync.dma_start(
                labf[:, lab_off : lab_off + lg],
                lab_ext[:, lab_off : lab_off + lg],
            )
            lab_off += lg
            if next_dma < 2:
                emit_pred_dma(next_dma)
                next_dma += 1

        # ---- PE warmup on memset data during the DMA fill.
        nc.gpsimd.memset(warm_t[:], 1.0)
        nc.vector.memset(outt[:], 0.0)
        for w in range(WARM_MMS):
            nc.tensor.matmul(
                warm_ps[:64, :],
                warm_t[:, :64],
                warm_t[:, :128],
                start=(w == 0),
                stop=(w == WARM_MMS - 1),
            )

        # ---- main loop over one-hot chunks -------------------------------
        u = 0
        for ci in range(NCHUNKS):
            coff = ci * FC
            while next_dma < NGROUPS and next_dma * FG < coff + 3 * FC:
                emit_pred_dma(next_dma)
                next_dma += 1

            oh = oh_pool.tile([128, NG * FC], fp16, tag="oh")
            oh2 = oh.rearrange("p (g t) -> p g t", g=NG)
            lab_sl = labf[:, coff : coff + FC]
            for g in range(DVE_K):
                nc.vector.tensor_scalar(
                    oh2[:, g, :],
                    lab_sl,
                    float(g + 1),
                    None,
                    mybir.AluOpType.is_equal,
                )
            for r in range(NRAMP):
                nc.scalar.activation(
                    oh2[:, DVE_K + r, :],
                    lab_sl,
                    mybir.ActivationFunctionType.Abs,
                    bias=bias_t[:, r : r + 1],
                )

            for uc in range(FC // QB):
                g = u // (FG // QB)
                ug = u % (FG // QB)
                t = u % NTILE
                nc.tensor.matmul(
                    psums[t][32 * t : 32 * t + NSTAT, :NMOV],
                    slabs[g][:, ug * NSTAT : (ug + 1) * NSTAT],
                    oh2[:, :, uc * QB : (uc + 1) * QB],
                    start=(u < NTILE),
                    stop=(u >= NUNITS - NTILE),
                    tile_position=(0, 32 * t),
                    skip_group_check=True,
                )
                u += 1

        # ---- output ------------------------------------------------------
        for t in range(NTILE):
            eng = nc.vector if t % 2 == 0 else nc.scalar
            if t % 2 == 0:
                nc.vector.tensor_copy(
                    outt[32 * t : 32 * t + NSTAT, :NMOV],
                    psums[t][32 * t : 32 * t + NSTAT, :NMOV],
                )
            else:
                nc.scalar.activation(
                    outt[32 * t : 32 * t + NSTAT, :NMOV],
                    psums[t][32 * t : 32 * t + NSTAT, :NMOV],
                    mybir.ActivationFunctionType.Copy,
                )
        # keep the warm matmuls live (scratch cols, 32-aligned psum base)
        nc.vector.tensor_copy(outt[32:33, NMOV:], warm_ps[32:33, :8])
        nc.sync.dma_start(out_ext[:], outt[:])
    nc.compile()
    return nc


@functools.lru_cache(maxsize=1)
def _get_program():
    return build_nc()


def pack_core(pred_core, labels_core):
    """Host-side packing into the kernel's DMA layouts.

    pred -> fp16 * 2^14 in group-major stationary slabs
    [g, p, (u, b, c)] with the ones column baked in at c == 8;
    labels -> bf16 [p, t].  Pixel (p, t) = core_linear[p * FTOT + t].
    """
    import ml_dtypes

    ph = (
        np.asarray(pred_core, dtype=np.float32).reshape(C, 128, FTOT)
        * np.float32(PRED_SCALE)
    ).astype(np.float16)
    arr = np.empty((128, FTOT, NCH), dtype=np.float16)
    arr[:, :, :C] = ph.transpose(1, 2, 0)
    arr[:, :, C] = np.float16(1.0)
    pred_r = np.ascontiguousarray(
        arr.reshape(128, NGROUPS, FG * NCH).transpose(1, 0, 2)
    ).reshape(NGROUPS * 128, FG // QB * NSTAT)
    lab_r = labels_core.reshape(128, FTOT).astype(ml_dtypes.bfloat16)
    return pred_r, lab_r


def make_in_maps(pred_flat, labels_flat, pcore=PCORE, ncores=NCORES):
    in_maps = []
    for i in range(ncores):
        sl = slice(i * pcore, (i + 1) * pcore)
        pred_r, lab_r = pack_core(pred_flat[:, sl], labels_flat[sl])
        in_maps.append({"pred": pred_r, "labels": lab_r})
    return in_maps


def extract_SN(res_core):
    """From one core's outputs: S_scaled [C, K] and N [K].

    Moving groups 0..25 are direct one-hots (labels 1..26); groups
    26..33 are abs-ramps |lab - r| with centers r = 26..33, whose
    second difference recovers labels 27..32 (and their counts from
    the ones row).
    """
    ps = res_core["out_s"].astype(np.float64)[:, :NMOV]
    d = np.zeros((NCH, NG))
    for t in range(4):
        r = ps[32 * t : 32 * t + NSTAT, :].reshape(QB, NCH, NG, QB)
        d += r[np.arange(QB), :, :, np.arange(QB)].sum(axis=0)  # [NCH, NG]
    S = np.zeros((C, K))
    N = np.zeros(K)
    S[:, :DVE_K] = d[:C, :DVE_K]
    N[:DVE_K] = d[C, :DVE_K]
    A = d[:, DVE_K:]  # [NCH, 7], centers 27..33
    for k in range(DVE_K + 1, K + 1):  # labels 28..32
        c = k - DVE_K  # 1..5
        S[:, k - 1] = (A[:C, c - 1] - 2 * A[:C, c] + A[:C, c + 1]) / 2
        N[k - 1] = (A[C, c - 1] - 2 * A[C, c] + A[C, c + 1]) / 2
    return S, N


def finish_host(results, num_kernel):
    S = np.zeros((C, K))
    N = np.zeros(K)
    for r in results:
        Si, Ni = extract_SN(r)
        S += Si
        N += Ni
    S /= PRED_SCALE
    A = N * np.sum(S * S, axis=0)  # [K]
    kk = int(num_kernel)
    A = A[:kk]
    pair = A[:, None] + A[None, :]
    Dm = np.maximum(SIGMA_DIS - np.sqrt(pair), 0.0)
    term = np.log(Dm * Dm + 1.0)
    L = float(np.sum(np.triu(term, k=1)))
    L *= (kk - 1) / kk
    return np.float32(L)


_last_results = None


def kernel(pred_similarities, regions_mask, kernel_labels, num_kernel, **kw):
    global _last_results
    from concourse.bass_utils import run_bass_kernel_spmd

    pred_flat = np.asarray(pred_similarities, dtype=np.float32).reshape(C, PTOT)
    labels_flat = np.asarray(kernel_labels, dtype=np.int32).reshape(PTOT)

    nc = _get_program()
    in_maps = make_in_maps(pred_flat, labels_flat)
    res = run_bass_kernel_spmd(nc, in_maps, list(range(NCORES)))
    _last_results = res
    return finish_host(
        [res.results[i] for i in range(NCORES)], num_kernel
    )


# revision 15
# speedup vs baseline: 1.5160x; 1.5160x over previous
"""Trainium2 Bass kernel for nn_DiscriminationLoss (segment_reduce).

v6 design (8 NeuronCores, pixel-sharded; full inputs in, full loss out):

  - HOST packs pred into the PE stationary layout directly: fp16,
    pre-scaled by 2^14, block-diagonal unit slabs [p, (u, b, c)] with a
    baked-in ones column at c == 8 (per-kernel counts).  Numerically
    identical to the on-chip ScalarE cast the v2 kernel used, but
    removes that whole stage and halves pred DMA to 9.4 MB/core.  Each
    512-block-col DMA group is one contiguous 1.18 MB HBM read.
  - Labels ship as bf16 (0..32 exact).  fp16 tensor_scalar inputs take
    a ~13x slow path on DVE; bf16-in -> fp16-out is the fast combo.
  - One-hot is class-major [p, (g, t)]: every DVE tensor_scalar
    (is_equal) writes a dense step-1 16-bit AP (4x_2p mode, ~194 ns
    per [128, 512] op).  The matmul moving operand reads it via a 2D
    AP (g: stride FC, b: 4 dense) at the usual 1 col/cycle.
  - DVE alone cannot cover 32 classes under the DMA floor, and GpSimd
    tensor ops are banned (they serialize the DVE via an SBUF port
    mutex: DVE TS ops stall for the entire concurrent GpSimd op).  So
    ScalarE covers labels 28..32 via the RAMP trick: moving groups
    27..33 hold |lab - r| (one ACTIVATE Abs per ramp, exact in fp16),
    and the host recovers S[k] = (A[k-1] - 2A[k] + A[k+1]) / 2 -- the
    second difference of abs-ramps is exactly the one-hot.  Counts for
    those labels fall out of the ones row the same way.
  - 8 chunks of 512 block-cols keep the pipeline fill/tail short
    (chunk-0's one-hot serializes before the first real matmul).
  - PE: two concurrent 64-col tiles (t = u%2), separate PSUM banks,
    [128, 36]^T @ [128, 136] per unit, PSUM-accumulated over 512 units
    per tile.
  - 36 warmup matmuls trip the PE HAM clock gate during the DMA fill
    (more would block the real stream: the Tensor queue is in-order
    and cold MMs run 107 ns apiece).
  - Host sums per-core partials (the "psum" step of the sharding hint)
    and evaluates the tiny O(K^2) pairwise tail in f64.
"""

import sys
import functools

sys.path.insert(0, "/opt/trn_rl_repo")

import numpy as np

C = 8
K = 32
NCORES = 8
H = W = 2048
PTOT = H * W
PCORE = PTOT // NCORES  # 524288
SIGMA_DIS = 3.0
PRED_SCALE = float(2.0**14)

QB = 4            # pixel-blocks per matmul unit (block-diagonal trick)
NCH = C + 1       # 8 pred channels + ones column (counts)
NSTAT = NCH * QB  # stationary columns per unit = 36 (fits the 64-col tile)
NTILE = 2         # concurrent 64-col PE tiles
DVE_K = 26        # direct one-hot classes (labels 1..26) on DVE
NRAMP = 8         # abs-ramp groups, centers 26..33, on ScalarE
NG = DVE_K + NRAMP  # moving groups per unit = 34
NMOV = NG * QB    # moving columns per unit = 136
FTOT = PCORE // 128  # 4096 block-cols
FG = 512          # block-cols per pred DMA group (1.18 MB contiguous)
CS = (512, 1024, 1024, 1024, 512)  # one-hot chunk sizes (block-cols)
FCMAX = max(CS)
NGROUPS = FTOT // FG
NUNITS = FTOT // QB
SLAB_BUFS = 6
WARM_MMS = 36
LAB_SIZES = (512, 512, 1024, 2048)  # label DMA split (block-cols)


def build_nc():
    import concourse.bacc as bacc
    import concourse.tile as tile
    import concourse.mybir as mybir
    from contextlib import ExitStack

    f32 = mybir.dt.float32
    fp16 = mybir.dt.float16
    bf16 = mybir.dt.bfloat16

    nc = bacc.Bacc(
        "TRN2", target_bir_lowering=False, debug=False, num_devices=NCORES
    )
    # Group-major pred slabs: row block g*128..g*128+127 is DMA group g,
    # a single contiguous 1.18 MB HBM region.
    pred_ext = nc.dram_tensor(
        "pred", [NGROUPS * 128, FG // QB * NSTAT], fp16, kind="ExternalInput"
    )
    lab_ext = nc.dram_tensor("labels", [128, FTOT], bf16, kind="ExternalInput")
    out_ext = nc.dram_tensor(
        "out_s", [128, NMOV + 8], f32, kind="ExternalOutput"
    )

    with tile.TileContext(nc) as tc, ExitStack() as ctx:
        const_pool = ctx.enter_context(tc.tile_pool(name="const", bufs=1))
        slab_pool = ctx.enter_context(
            tc.tile_pool(name="slab", bufs=SLAB_BUFS)
        )
        oh_pool = ctx.enter_context(tc.tile_pool(name="oh", bufs=2))
        psum_pool = ctx.enter_context(
            tc.tile_pool(name="psum", bufs=1, space="PSUM")
        )

        labf = const_pool.tile([128, FTOT], bf16)
        outt = const_pool.tile([128, NMOV + 8], f32)
        warm_t = const_pool.tile([128, 128], bf16)
        # ramp bias constants: -(center) for centers 27..33
        bias_t = const_pool.tile([128, NRAMP], f32)
        for i in range(NRAMP):
            nc.vector.memset(bias_t[:, i : i + 1], -float(DVE_K + i))

        psums = [
            psum_pool.tile([128, 512], f32, name=f"psum_t{t}")
            for t in range(NTILE)
        ]  # one full PSUM bank per PE tile
        warm_ps = psum_pool.tile([128, 128], f32)

        # ---- DMA: labels + pred interleaved on the sync ring, labels
        # first so chunk-0's one-hot can start ASAP.
        slabs = []

        def emit_pred_dma(g):
            st = slab_pool.tile([128, FG // QB * NSTAT], fp16, tag="slab")
            nc.sync.dma_start(st, pred_ext[g * 128 : (g + 1) * 128, :])
            slabs.append(st)

        lab_off = 0
        next_dma = 0
        for lg in LAB_SIZES:
            nc.sync.dma_start(
                labf[:, lab_off : lab_off + lg],
                lab_ext[:, lab_off : lab_off + lg],
            )
            lab_off += lg
            if next_dma < 2:
                emit_pred_dma(next_dma)
                next_dma += 1

        # ---- PE warmup on memset data during the DMA fill.
        nc.gpsimd.memset(warm_t[:], 1.0)
        nc.vector.memset(outt[:], 0.0)
        for w in range(WARM_MMS):
            nc.tensor.matmul(
                warm_ps[:64, :],
                warm_t[:, :64],
                warm_t[:, :128],
                start=(w == 0),
                stop=(w == WARM_MMS - 1),
            )

        # ---- main loop over one-hot chunks -------------------------------
        u = 0
        coff = 0
        for ci, fcc in enumerate(CS):
            while next_dma < NGROUPS and next_dma * FG < coff + fcc + 2048:
                emit_pred_dma(next_dma)
                next_dma += 1

            oht = oh_pool.tile([128, NG * FCMAX], fp16, tag="oh")
            oh2 = oht[:, : NG * fcc].rearrange("p (g t) -> p g t", g=NG)
            lab_sl = labf[:, coff : coff + fcc]
            for g in range(DVE_K):
                nc.vector.tensor_scalar(
                    oh2[:, g, :],
                    lab_sl,
                    float(g + 1),
                    None,
                    mybir.AluOpType.is_equal,
                )
            for r in range(NRAMP):
                nc.scalar.activation(
                    oh2[:, DVE_K + r, :],
                    lab_sl,
                    mybir.ActivationFunctionType.Abs,
                    bias=bias_t[:, r : r + 1],
                )

            for uc in range(fcc // QB):
                g = u // (FG // QB)
                ug = u % (FG // QB)
                t = u % NTILE
                nc.tensor.matmul(
                    psums[t][64 * t : 64 * t + NSTAT, :NMOV],
                    slabs[g][:, ug * NSTAT : (ug + 1) * NSTAT],
                    oh2[:, :, uc * QB : (uc + 1) * QB],
                    start=(u < NTILE),
                    stop=(u >= NUNITS - NTILE),
                    tile_position=(0, 64 * t),
                    skip_group_check=True,
                )
                u += 1
            coff += fcc

        # ---- output ------------------------------------------------------
        nc.vector.tensor_copy(
            outt[:NSTAT, :NMOV], psums[0][:NSTAT, :NMOV]
        )
        nc.scalar.activation(
            outt[64 : 64 + NSTAT, :NMOV],
            psums[1][64 : 64 + NSTAT, :NMOV],
            mybir.ActivationFunctionType.Copy,
        )
        # keep the warm matmuls live (scratch cols, 32-aligned psum base)
        nc.vector.tensor_copy(outt[32:33, NMOV:], warm_ps[32:33, :8])
        nc.sync.dma_start(out_ext[:], outt[:])
    nc.compile()
    return nc


@functools.lru_cache(maxsize=1)
def _get_program():
    return build_nc()


def pack_core(pred_core, labels_core):
    """Host-side packing into the kernel's DMA layouts.

    pred -> fp16 * 2^14 in group-major stationary slabs
    [g, p, (u, b, c)] with the ones column baked in at c == 8;
    labels -> bf16 [p, t].  Pixel (p, t) = core_linear[p * FTOT + t].
    """
    import ml_dtypes

    ph = (
        np.asarray(pred_core, dtype=np.float32).reshape(C, 128, FTOT)
        * np.float32(PRED_SCALE)
    ).astype(np.float16)
    arr = np.empty((128, FTOT, NCH), dtype=np.float16)
    arr[:, :, :C] = ph.transpose(1, 2, 0)
    arr[:, :, C] = np.float16(1.0)
    pred_r = np.ascontiguousarray(
        arr.reshape(128, NGROUPS, FG * NCH).transpose(1, 0, 2)
    ).reshape(NGROUPS * 128, FG // QB * NSTAT)
    lab_r = labels_core.reshape(128, FTOT).astype(ml_dtypes.bfloat16)
    return pred_r, lab_r


def make_in_maps(pred_flat, labels_flat, pcore=PCORE, ncores=NCORES):
    in_maps = []
    for i in range(ncores):
        sl = slice(i * pcore, (i + 1) * pcore)
        pred_r, lab_r = pack_core(pred_flat[:, sl], labels_flat[sl])
        in_maps.append({"pred": pred_r, "labels": lab_r})
    return in_maps


def extract_SN(res_core):
    """From one core's outputs: S_scaled [C, K] and N [K].

    Moving groups 0..25 are direct one-hots (labels 1..26); groups
    26..33 are abs-ramps |lab - r| with centers r = 26..33, whose
    second difference recovers labels 27..32 (and their counts from
    the ones row).
    """
    ps = res_core["out_s"].astype(np.float64)[:, :NMOV]
    d = np.zeros((NCH, NG))
    for t in range(2):
        r = ps[64 * t : 64 * t + NSTAT, :].reshape(QB, NCH, NG, QB)
        d += r[np.arange(QB), :, :, np.arange(QB)].sum(axis=0)  # [NCH, NG]
    S = np.zeros((C, K))
    N = np.zeros(K)
    S[:, :DVE_K] = d[:C, :DVE_K]
    N[:DVE_K] = d[C, :DVE_K]
    A = d[:, DVE_K:]  # [NCH, 7], centers 27..33
    for k in range(DVE_K + 1, K + 1):  # labels 28..32
        c = k - DVE_K  # 1..5
        S[:, k - 1] = (A[:C, c - 1] - 2 * A[:C, c] + A[:C, c + 1]) / 2
        N[k - 1] = (A[C, c - 1] - 2 * A[C, c] + A[C, c + 1]) / 2
    return S, N


def finish_host(results, num_kernel):
    S = np.zeros((C, K))
    N = np.zeros(K)
    for r in results:
        Si, Ni = extract_SN(r)
        S += Si
        N += Ni
    S /= PRED_SCALE
    A = N * np.sum(S * S, axis=0)  # [K]
    kk = int(num_kernel)
    A = A[:kk]
    pair = A[:, None] + A[None, :]
    Dm = np.maximum(SIGMA_DIS - np.sqrt(pair), 0.0)
    term = np.log(Dm * Dm + 1.0)
    L = float(np.sum(np.triu(term, k=1)))
    L *= (kk - 1) / kk
    return np.float32(L)


_last_results = None


def kernel(pred_similarities, regions_mask, kernel_labels, num_kernel, **kw):
    global _last_results
    from concourse.bass_utils import run_bass_kernel_spmd

    pred_flat = np.asarray(pred_similarities, dtype=np.float32).reshape(C, PTOT)
    labels_flat = np.asarray(kernel_labels, dtype=np.int32).reshape(PTOT)

    nc = _get_program()
    in_maps = make_in_maps(pred_flat, labels_flat)
    res = run_bass_kernel_spmd(nc, in_maps, list(range(NCORES)))
    _last_results = res
    return finish_host(
        [res.results[i] for i in range(NCORES)], num_kernel
    )
